# revision 1
# baseline (speedup 1.0000x reference)
"""HashEmbedding (hash -> gather -> sum-pool) on 8 TRN2 NeuronCores.

Strategy: batch-data-parallel (each core owns 512 of the 4096 batch rows
and a full copy of the [1M, 128] f32 table in its local HBM). Per-core
gather traffic (512*200 rows x 512 B = 52.4 MB) matches vocab-sharding
but needs no collectives.

The gather primitive with sim/HW parity on this stack is the ANT
`dma_gather` (gpsimd SWDGE CounterMachine, 16 SDMA engines). Its indices
are int16, so a single call can only address a 32768-row table window.
The host therefore hashes the ids (numpy uint32, exact) and sorts each
core's 102,400 (batch,slot) positions by window; the device executes 31
fixed-shape window gathers (capacity-padded) and pools with the
TensorEngine: per gathered chunk of 128 rows, a 0/1 assignment matrix
A[p, m] = (slot[p] == m) is built on the DVE via is_equal against an
iota, and psum[m, d] += A^T @ G accumulates the sum-pool. Padding slots
are -1 so padded rows match no column and contribute zero. Four PSUM
banks hold the four 128-row batch groups per core; results copy out
through SBUF.
"""

import sys

if "/opt/trn_rl_repo" not in sys.path:
    sys.path.insert(0, "/opt/trn_rl_repo")

import numpy as np

B, H, D, V = 4096, 200, 128, 1_000_000
NCORES = 8
BPC = B // NCORES              # 512 batch rows per core
NPASS = 4                      # batch groups of 128 rows (PSUM M limit)
WBITS = 15
W = 1 << WBITS                 # 32768-row window (int16 index limit)
NW = (V + W - 1) // W          # 31 windows
CAP = 1024                     # capacity per (window, pass); mu=839, sigma=28
CALL_IDX = NPASS * CAP         # 4096 indices per window call
CHUNKS = CALL_IDX // 128       # 32 matmul chunks per call

_cache: dict = {}


def _host_prep(x_core):
    """Hash + window-sort one core's ids -> (loc16 [NW,128,256] wrapped,
    slotf [NW,128,CHUNKS] f32)."""
    idx = (
        (x_core.astype(np.uint32).ravel() * np.uint32(2654435761))
        % np.uint32(V)
    ).astype(np.int32)                       # [BPC*H]
    b = np.repeat(np.arange(BPC, dtype=np.int32), H)
    win = idx >> WBITS
    loc = idx & (W - 1)
    grp = b >> 7                              # pass
    slot = b & 127

    bucket = win * NPASS + grp
    order = np.argsort(bucket, kind="stable")
    bs, ls, ss = bucket[order], loc[order], slot[order]
    counts = np.bincount(bucket, minlength=NW * NPASS)
    if counts.max() > CAP:
        raise RuntimeError(f"window bucket overflow: {counts.max()} > {CAP}")
    starts = np.zeros(NW * NPASS, dtype=np.int64)
    starts[1:] = np.cumsum(counts)[:-1]
    rank = np.arange(bs.size) - starts[bs]

    loc_arr = np.zeros((NW, NPASS, CAP), dtype=np.int16)
    slot_arr = np.full((NW, NPASS, CAP), -1.0, dtype=np.float32)
    loc_arr[bs // NPASS, bs % NPASS, rank] = ls.astype(np.int16)
    slot_arr[bs // NPASS, bs % NPASS, rank] = ss.astype(np.float32)

    flat_loc = loc_arr.reshape(NW, CALL_IDX)
    # SWDGE wrapped layout: position i at [partition i%16, col i//16],
    # replicated to all 8 Q7-core partition groups.
    wrapped = flat_loc.reshape(NW, CALL_IDX // 16, 16).transpose(0, 2, 1)
    loc16 = np.tile(wrapped, (1, 8, 1)).copy()            # [NW, 128, 256]
    # slot layout matching gather output: position i -> (p=i%128, c=i//128)
    slotf = (
        slot_arr.reshape(NW, CHUNKS, 128).transpose(0, 2, 1).copy()
    )                                                      # [NW, 128, CHUNKS]
    return loc16, slotf


def _build():
    import concourse.tile as tile
    from concourse import bacc, mybir

    i16, i32, f32 = mybir.dt.int16, mybir.dt.int32, mybir.dt.float32
    Alu = mybir.AluOpType

    nc = bacc.Bacc(
        "TRN2",
        target_bir_lowering=False,
        debug=False,
        enable_asserts=False,
        # SWDGE descriptor carveout: a dma_gather call of N descriptors
        # needs >= 32*N bytes here (HW-verified: 512 ok / 1024 crash at
        # the 16384 default; 1024 ok / 2048 crash at 32768).
        dynamic_dma_scratch_size=32768,
    )
    tb_ap = nc.dram_tensor("table", [NW * W, D], f32, kind="ExternalInput").ap()
    loc_ap = nc.dram_tensor(
        "loc16", [NW, 128, CALL_IDX // 16], i16, kind="ExternalInput"
    ).ap()
    slot_ap = nc.dram_tensor(
        "slotf", [NW, 128, CHUNKS], f32, kind="ExternalInput"
    ).ap()
    out_ap = nc.dram_tensor("out", [BPC, D], f32, kind="ExternalOutput").ap()

    with tile.TileContext(nc) as tc:
        with (
            tc.tile_pool(name="iop", bufs=1) as iop,
            tc.tile_pool(name="inp", bufs=3) as inp,
            tc.tile_pool(name="gp", bufs=3) as gp,
            tc.tile_pool(name="ap_", bufs=3) as ap_,
            tc.tile_pool(name="op", bufs=2) as op,
            tc.tile_pool(name="pp", bufs=1, space="PSUM") as pp,
        ):
            iota_i = iop.tile([128, 128], i32, name="iota_i")
            nc.gpsimd.iota(iota_i[:], [[1, 128]], base=0, channel_multiplier=0)
            iota_f = iop.tile([128, 128], f32, name="iota_f")
            nc.vector.tensor_copy(iota_f[:], iota_i[:])

            psums = [
                pp.tile([128, D], f32, name=f"ps{g}", tag=f"ps{g}")
                for g in range(NPASS)
            ]

            SUBC = CAP // 128                     # 8 chunks per (window, pass)
            for w in range(NW):
                lt = inp.tile([128, CALL_IDX // 16], i16, name="lt", tag="lt")
                nc.sync.dma_start(out=lt[:], in_=loc_ap[w])
                st = inp.tile([128, CHUNKS], f32, name="st", tag="st")
                nc.sync.dma_start(out=st[:], in_=slot_ap[w])

                A = ap_.tile([128, CHUNKS, 128], f32, name="A", tag="A")
                iota_bc = iota_f[:].unsqueeze(1).broadcast_to([128, CHUNKS, 128])
                st_bc = st[:].unsqueeze(2).broadcast_to([128, CHUNKS, 128])
                nc.vector.tensor_tensor(A[:], iota_bc, st_bc, Alu.is_equal)

                for grp in range(NPASS):
                    # one 1024-descriptor gather per (window, batch group)
                    g = gp.tile([128, SUBC, D], f32, name="g", tag="g")
                    nc.gpsimd.dma_gather(
                        g[:],
                        tb_ap[w * W : (w + 1) * W, :],
                        lt[:, grp * (CAP // 16) : (grp + 1) * (CAP // 16)],
                        CAP,
                        CAP,
                        D,
                    )
                    for c in range(SUBC):
                        nc.tensor.matmul(
                            psums[grp][:],
                            A[:, grp * SUBC + c, :],
                            g[:, c, :],
                            start=(w == 0 and c == 0),
                            stop=(w == NW - 1 and c == SUBC - 1),
                        )

            for grp in range(NPASS):
                outs = op.tile([128, D], f32, name="outs", tag="outs")
                nc.vector.tensor_copy(outs[:], psums[grp][:])
                nc.sync.dma_start(
                    out=out_ap[grp * 128 : (grp + 1) * 128, :], in_=outs[:]
                )

    nc.compile()
    return nc


def _run(x, table, trace=False):
    from concourse.bass_utils import run_bass_kernel_spmd

    if "nc" not in _cache:
        _cache["nc"] = _build()
    nc = _cache["nc"]

    x_np = np.asarray(x)
    # pad the table to NW*W rows so every gather window is a full 32768
    tb = np.zeros((NW * W, D), dtype=np.float32)
    tb[:V] = np.asarray(table, dtype=np.float32)
    in_maps = []
    for c in range(NCORES):
        loc16, slotf = _host_prep(x_np[c * BPC : (c + 1) * BPC])
        in_maps.append({"table": tb, "loc16": loc16, "slotf": slotf})
    res = run_bass_kernel_spmd(nc, in_maps, list(range(NCORES)), trace=trace)
    out = np.concatenate(
        [res.results[c]["out"] for c in range(NCORES)], axis=0
    ).astype(np.float32)
    return out, res


def kernel(x, table):
    out, _ = _run(x, table, trace=False)
    return out



# revision 5
# speedup vs baseline: 3.0253x; 3.0253x over previous
"""HashEmbedding (hash -> gather -> sum-pool) on 8 TRN2 NeuronCores.

Strategy: batch-data-parallel (each core owns 512 of the 4096 batch rows
and a full copy of the table in its local HBM). Per-core gather traffic
is 512*200 = 102,400 rows; no collectives.

The gather primitive is the ANT `dma_gather` (SWDGE CounterMachine).
Profiling the single-queue f32 baseline showed GpSimd (the Q7 pair doing
descriptor generation) 97% busy at 1.18 ms while DMA/PE idled, so this
version attacks descriptor generation and the fp32 matmul rate:

 1. 4 SWDGE queues: the Q7 ucode serves queue q with core pair (2q,
    2q+1), so round-robining the (window, batch-group) gather calls over
    queue_num 0-3 runs descriptor generation on 4 core pairs
    concurrently instead of 1.
 2. bf16 table: the host converts the f32 table to bf16 once. Gathered
    rows shrink 512B -> 256B (half the HBM traffic) and the pooling
    matmuls run at full bf16 PE rate instead of 1/4 fp32 rate.
 3. Exact-count gathers: window-bucket capacity padding (CAP=1024 vs
    mean 839) is marked with trailing -1 indices, which the Q7 ucode
    trims before generating descriptors; the per-call valid count is
    read at runtime from a counts tensor into a Pool register
    (num_idxs_reg), as the sequencer-side ring accounting requires.
    This skips ~18% of descriptors and gather bytes.

Pooling is unchanged: per gathered chunk of 128 rows, a 0/1 assignment
matrix A[p, m] = (slot[p] == m) is built on the DVE (bf16), and
psum[m, d] += A^T @ G accumulates the sum-pool across windows. Padding
slots are -1 so they match no column. Gather tiles are memset once per
pool buffer (first two windows) so skipped tail positions never hold
NaN bit patterns (0 * NaN would poison the psum).
"""

import sys

if "/opt/trn_rl_repo" not in sys.path:
    sys.path.insert(0, "/opt/trn_rl_repo")

import numpy as np

B, H, D, V = 4096, 200, 128, 1_000_000
NCORES = 8
BPC = B // NCORES              # 512 batch rows per core
NPASS = 4                      # batch groups of 128 rows (PSUM M limit)
WBITS = 15
W = 1 << WBITS                 # 32768-row window (int16 index limit)
NW = (V + W - 1) // W          # 31 windows
CAP = 1024                     # capacity per (window, pass); mu=839, sigma=28
CALL_IDX = NPASS * CAP         # 4096 indices per window
CHUNKS = CALL_IDX // 128       # 32 matmul chunks per window
SUBC = CAP // 128              # 8 chunks per (window, pass) call
NQ = 4                         # SWDGE queues (ucode MAX_SWDGE_QUEUES)

_cache: dict = {}


def _f32_to_bf16(a):
    """Round-to-nearest-even f32 -> bf16, as uint16 bits (pure numpy)."""
    b = np.ascontiguousarray(a, dtype=np.float32).view(np.uint32)
    rounding = np.uint32(0x7FFF) + ((b >> np.uint32(16)) & np.uint32(1))
    return ((b + rounding) >> np.uint32(16)).astype(np.uint16)


def _host_prep(x_core):
    """Hash + window-sort one core's ids.

    Returns (loc16 [NW,128,256] wrapped with -1 tail padding,
             slotf [NW,128,CHUNKS] f32 with -1 padding,
             counts [1, NW*NPASS] int32 exact per-call valid counts)."""
    idx = (
        (x_core.astype(np.uint32).ravel() * np.uint32(2654435761))
        % np.uint32(V)
    ).astype(np.int32)                       # [BPC*H]
    b = np.repeat(np.arange(BPC, dtype=np.int32), H)
    win = idx >> WBITS
    loc = idx & (W - 1)
    grp = b >> 7                              # pass
    slot = b & 127

    bucket = win * NPASS + grp
    order = np.argsort(bucket, kind="stable")
    bs, ls, ss = bucket[order], loc[order], slot[order]
    counts = np.bincount(bucket, minlength=NW * NPASS)
    if counts.max() > CAP:
        raise RuntimeError(f"window bucket overflow: {counts.max()} > {CAP}")
    if counts.min() < 1:
        raise RuntimeError("empty (window, pass) bucket")
    starts = np.zeros(NW * NPASS, dtype=np.int64)
    starts[1:] = np.cumsum(counts)[:-1]
    rank = np.arange(bs.size) - starts[bs]

    loc_arr = np.full((NW, NPASS, CAP), -1, dtype=np.int16)
    slot_arr = np.full((NW, NPASS, CAP), -1.0, dtype=np.float32)
    loc_arr[bs // NPASS, bs % NPASS, rank] = ls.astype(np.int16)
    slot_arr[bs // NPASS, bs % NPASS, rank] = ss.astype(np.float32)

    flat_loc = loc_arr.reshape(NW, CALL_IDX)
    # SWDGE wrapped layout: within a (window, pass) call, position i sits
    # at [partition i%16, col grp*(CAP//16) + i//16], replicated to all 8
    # Q7-core partition groups (each queue's core pair reads its own).
    wrapped = flat_loc.reshape(NW, NPASS, CAP // 16, 16).transpose(0, 3, 1, 2)
    wrapped = wrapped.reshape(NW, 16, CALL_IDX // 16)
    loc16 = np.tile(wrapped, (1, 8, 1)).copy()            # [NW, 128, 256]
    # slot layout matching gather output: position i -> (p=i%128, c=i//128)
    slotf = (
        slot_arr.reshape(NW, CHUNKS, 128).transpose(0, 2, 1).copy()
    )                                                      # [NW, 128, CHUNKS]
    cnts = counts.astype(np.int32).reshape(1, NW * NPASS)
    return loc16, slotf, cnts


def _build():
    import concourse.tile as tile
    from concourse import bacc, mybir

    i16, i32, f32 = mybir.dt.int16, mybir.dt.int32, mybir.dt.float32
    bf16 = mybir.dt.bfloat16
    Alu = mybir.AluOpType

    nc = bacc.Bacc(
        "TRN2",
        target_bir_lowering=False,
        debug=False,
        enable_asserts=False,
        # SWDGE descriptor carveout: a dma_gather call of N descriptors
        # needs >= 32*N bytes here (HW-verified on the f32 baseline).
        # Queues write their rings to different partition groups, so the
        # same carveout serves all 4 queues.
        dynamic_dma_scratch_size=32768,
        num_swdge_queues=NQ,
    )
    tb_ap = nc.dram_tensor("table", [NW * W, D], bf16, kind="ExternalInput").ap()
    loc_ap = nc.dram_tensor(
        "loc16", [NW, 128, CALL_IDX // 16], i16, kind="ExternalInput"
    ).ap()
    slot_ap = nc.dram_tensor(
        "slotf", [NW, 128, CHUNKS], f32, kind="ExternalInput"
    ).ap()
    cnt_ap = nc.dram_tensor(
        "counts", [1, NW * NPASS], i32, kind="ExternalInput"
    ).ap()
    out_ap = nc.dram_tensor("out", [BPC, D], f32, kind="ExternalOutput").ap()

    with tile.TileContext(nc) as tc:
        with (
            tc.tile_pool(name="iop", bufs=1) as iop,
            tc.tile_pool(name="inp", bufs=4) as inp,
            tc.tile_pool(name="gp", bufs=8) as gp,
            tc.tile_pool(name="ap_", bufs=3) as ap_,
            tc.tile_pool(name="op", bufs=2) as op,
            tc.tile_pool(name="pp", bufs=1, space="PSUM") as pp,
        ):
            iota_i = iop.tile([128, 128], i32, name="iota_i")
            nc.gpsimd.iota(iota_i[:], [[1, 128]], base=0, channel_multiplier=0)
            iota_f = iop.tile([128, 128], f32, name="iota_f")
            nc.vector.tensor_copy(iota_f[:], iota_i[:])

            ct = iop.tile([1, NW * NPASS], i32, name="ct")
            nc.sync.dma_start(out=ct[:], in_=cnt_ap[:, :])
            creg = nc.gpsimd.alloc_register("gather_cnt")

            psums = [
                pp.tile([128, D], f32, name=f"ps{g}", tag=f"ps{g}")
                for g in range(NPASS)
            ]

            for w in range(NW):
                lt = inp.tile([128, CALL_IDX // 16], i16, name="lt", tag="lt")
                nc.sync.dma_start(out=lt[:], in_=loc_ap[w])
                st = inp.tile([128, CHUNKS], f32, name="st", tag="st")
                nc.sync.dma_start(out=st[:], in_=slot_ap[w])

                A = ap_.tile([128, CHUNKS, 128], bf16, name="A", tag="A")
                iota_bc = iota_f[:].unsqueeze(1).broadcast_to([128, CHUNKS, 128])
                st_bc = st[:].unsqueeze(2).broadcast_to([128, CHUNKS, 128])
                nc.vector.tensor_tensor(A[:], iota_bc, st_bc, Alu.is_equal)

                for grp in range(NPASS):
                    # one gather per (window, batch group), queue = grp so
                    # the 4 Q7 core pairs generate descriptors in parallel
                    g = gp.tile([128, SUBC, D], bf16, name="g", tag="g")
                    if w < 2:
                        # first touch of each of the 8 pool buffers: clear
                        # boot garbage so trimmed tail positions are finite
                        nc.vector.memset(g[:], 0)
                    ci = w * NPASS + grp
                    nc.gpsimd.reg_load(creg, ct[0:1, ci : ci + 1])
                    nc.gpsimd.dma_gather(
                        g[:],
                        tb_ap[w * W : (w + 1) * W, :],
                        lt[:, grp * (CAP // 16) : (grp + 1) * (CAP // 16)],
                        CAP,
                        creg,
                        D,
                        queue_num=grp,
                    )
                    for c in range(SUBC):
                        nc.tensor.matmul(
                            psums[grp][:],
                            A[:, grp * SUBC + c, :],
                            g[:, c, :],
                            start=(w == 0 and c == 0),
                            stop=(w == NW - 1 and c == SUBC - 1),
                        )

            for grp in range(NPASS):
                outs = op.tile([128, D], f32, name="outs", tag="outs")
                nc.vector.tensor_copy(outs[:], psums[grp][:])
                nc.sync.dma_start(
                    out=out_ap[grp * 128 : (grp + 1) * 128, :], in_=outs[:]
                )

    nc.compile()
    return nc


def _run(x, table, trace=False):
    from concourse.bass_utils import run_bass_kernel_spmd

    if "nc" not in _cache:
        _cache["nc"] = _build()
    nc = _cache["nc"]

    x_np = np.asarray(x)
    # pad the table to NW*W rows so every gather window is a full 32768,
    # converted to bf16 (rel tolerance is 2e-2; bf16 error ~4e-3)
    import ml_dtypes

    tb = np.zeros((NW * W, D), dtype=np.uint16)
    tb[:V] = _f32_to_bf16(np.asarray(table, dtype=np.float32))
    tb16 = tb.view(ml_dtypes.bfloat16)
    in_maps = []
    for c in range(NCORES):
        loc16, slotf, cnts = _host_prep(x_np[c * BPC : (c + 1) * BPC])
        in_maps.append(
            {"table": tb16, "loc16": loc16, "slotf": slotf, "counts": cnts}
        )
    res = run_bass_kernel_spmd(nc, in_maps, list(range(NCORES)), trace=trace)
    out = np.concatenate(
        [res.results[c]["out"] for c in range(NCORES)], axis=0
    ).astype(np.float32)
    return out, res


def kernel(x, table):
    out, _ = _run(x, table, trace=False)
    return out
